# revision 37
# baseline (speedup 1.0000x reference)
"""Trainium2 Bass kernel for nn_Attn_66297115181215 (sparse_attention).

Reference computation (B=2, N=8192, C=256, H=8, Dh=C):
    qh/kh/vh = heads(emb @ W{q,k,v})            [B,H,N,Dh]
    attn = einsum("bhnd,bhne->bhde", qh, kh)    [B,H,Dh,Dh]
    attn = instance_norm(attn); attn = softmax(attn, axis=3)
    ctx  = einsum("bhde,bhne->bhdn", attn, vh)  [B,H,Dh,N]
    out  = ctx.transpose(0,3,2,1).reshape(B,N,C*H) @ Wo

Algebraic collapse: N only enters through G_b = emb_b^T emb_b [C,C]:
    A_h = Wq_h^T G Wk_h;  S_h = softmax(instnorm(A_h))
    out_b = emb_b @ P_b,  P_b = sum_h Wv_h (S_h^T Wo'_h)
(the softmax denominator is folded into Wo'_h rows).

Distribution (8 cores, no collectives): core c: b = c//4, j = c%4.
Every core redundantly computes G_b and the 8-head chain, then its own
N/4 slice of out_b (transposed; host transposes back).

Precision: inputs are cast host-side.  emb arrives as fp8(e4m3) and G
runs fp8 DoubleRow matmuls (fp32 accumulate).  The head chain and out
projection run bf16 (fp32 accumulate).  Output is written bf16 and
upcast on host.  End-to-end max-rel error ~6.5e-3 vs the 2e-2 gate.

Engine/latency structure:
 - Scalar/ACT stays on the single natural_log_exp table (rinv =
   exp(-0.5*ln(var+eps)); a table switch costs 1.3us).  The -mu shift
   cancels in softmax and is skipped.
 - The head chain is software-pipelined over head PAIRS with a lag of
   one pair, so each engine's in-order queue always has ready work:
   PH1 = A matmuls + stats -> rinv; PH2 = exp/softmax -> M -> P.
 - Warm-up matmuls burn the initial DMA wait so the PE p-state ramp
   (full clock only after ~3us of continuous work) is done before G.
 - DMA issue order = use order (emb pieces, Wk, Wq, Wv/Wo, embT).
"""

import os
import sys

sys.path.insert(0, "/opt/trn_rl_repo")

import ml_dtypes
import numpy as np

import concourse.bacc as bacc
import concourse.mybir as mybir
import concourse.tile as tile
from concourse.bass_utils import run_bass_kernel_spmd

# The kernel only uses Exp/Ln/Copy on the ACT engine, all of which live in
# the natural_log_exp_and_others table.  The default table-selection pass
# assigns Exp and Ln to different tables and thrashes (1283ns per load).
# Castrate every other table (preserving list positions, which are the
# act_func_set_id namespace) so one table serves the whole kernel.
_ORIG_GAT = bacc.get_activation_tables


def _single_act_table(arch):
    t = _ORIG_GAT(arch)
    return {name: (funcs if name == "natural_log_exp_and_others" else set())
            for name, funcs in t.items()}


bacc.get_activation_tables = _single_act_table

B, N, C, H = 2, 8192, 256, 8
EPS = 1e-5
NCORES = 8
CHUNK = N * B // NCORES          # 2048 out rows per core
T = N // 128                     # 64 row-blocks of emb

F32 = mybir.dt.float32
F32R = mybir.dt.float32r
BF16 = mybir.dt.bfloat16
F8 = mybir.dt.float8e4
AF = mybir.ActivationFunctionType
ALU = mybir.AluOpType
AX = mybir.AxisListType
PM = mybir.MatmulPerfMode

# emb piece sizes in t-blocks (all even so fp8 DoubleRow pairs never
# straddle pieces; big first so the tail G matmuls are tiny)
PIECES = [4, 8, 16, 16, 12, 6, 2]
assert sum(PIECES) == T and all(p % 2 == 0 for p in PIECES)


def build_kernel(dbg=False):
    nc = bacc.Bacc("TRN2", target_bir_lowering=False, debug=False,
                   num_devices=NCORES)
    dbg_t = {}
    if dbg:
        for nm, cols, dt in [("gdbg", 512, BF16), ("udbg", 4096, BF16),
                             ("sdbg", 32, F32), ("edbg", 16, F32),
                             ("rdbg", 16, F32), ("pdbg", 512, BF16)]:
            dbg_t[nm] = nc.dram_tensor(nm, [128, cols], dt,
                                       kind="ExternalOutput")

    # [p, t, c]: col t*C+c = emb row p*64+t (row-major reshape)
    emb8 = nc.dram_tensor("emb8", [128, T * C], F8, kind="ExternalInput")
    # [p, f, kc, n]: Wk[kc*128+p, f*512+n]
    wkt = nc.dram_tensor("wkt", [128, 8 * 512], BF16, kind="ExternalInput")
    # [p, h, kc, d]: Wq[kc*128+p, h*256+d]
    wqt = nc.dram_tensor("wqt", [128, 16 * C], BF16, kind="ExternalInput")
    # [p, h, s, x]: s=0,1 -> Wo[(s*128+p)*H+h, x]; s=2,3 -> Wv[x, h*C+(s-2)*128+p]
    wvo = nc.dram_tensor("wvo", [128, 32 * C], BF16, kind="ExternalInput")
    # [p, i, n]: embT row i*128+p of this core's chunk
    embt = nc.dram_tensor("embt", [128, 2 * CHUNK], BF16, kind="ExternalInput")
    outt = nc.dram_tensor("outt", [C, CHUNK], BF16, kind="ExternalOutput")

    with tile.TileContext(nc) as tc:
        with (
            tc.tile_pool(name="wbuf", bufs=1) as wbuf,
            tc.tile_pool(name="persist", bufs=1) as persist,
            tc.tile_pool(name="ebuf", bufs=1) as ebuf,
            tc.tile_pool(name="chain", bufs=1) as chain,
        ):
            # ---------------- DMAs, in use order ----------------
            emb_t = []
            off = 0
            for q, nb in enumerate(PIECES):
                s = persist.tile([128, nb * C], F8, name=f"e{q}")
                nc.sync.dma_start(s[:], emb8[:, off * C:(off + nb) * C])
                emb_t.append(s)
                off += nb
            wk_sb = wbuf.tile([128, 8 * 512], BF16, name="wk")
            nc.sync.dma_start(wk_sb[:], wkt[:])
            wq_sb = wbuf.tile([128, 16 * C], BF16, name="wq")
            nc.sync.dma_start(wq_sb[:], wqt[:])
            wvo_sb = wbuf.tile([128, 32 * C], BF16, name="wvo")
            nc.sync.dma_start(wvo_sb[:], wvo[:])
            embt_sb = wbuf.tile([128, 2 * CHUNK], BF16, name="embt")
            nc.sync.dma_start(embt_sb[:], embt[:])

            ones = persist.tile([128, 128], F32, name="ones")
            nc.gpsimd.memset(ones[:], 1.0)
            epst = persist.tile([128, 1], F32, name="epst")
            nc.gpsimd.memset(epst[:], EPS)
            # dummy activation: loads the (single) ACT table during the
            # initial DMA wait instead of at the first real Ln (1283ns
            # on the chain critical path otherwise)
            twarm = persist.tile([128, 1], F32, name="twarm")
            nc.scalar.activation(twarm[:], epst[:], AF.Exp)
            # st_sb per pair = [Ssum_h0, Ssum_h1, Ssq_h0, Ssq_h1]; the sum
            # cols hold per-partition (mean_even+mean_odd) = rowsum/256
            # over the 512-wide a_t tile
            scale4 = persist.tile([128, 4], F32, name="scale4")
            nc.gpsimd.memset(scale4[:, 0:2], 1.0 / 256.0)
            nc.gpsimd.memset(scale4[:, 2:4], 1.0 / float(C * C))
            # per-pair stats cols [4p..4p+3] = [s_h0, s_h1, q_h0, q_h1]
            statc = persist.tile([128, 2 * H], F32, name="statc")
            esum = persist.tile([128, 2 * H], F32, name="esum")
            rec = persist.tile([128, 2 * H], F32, name="rec")

            # ---------------- G = emb^T emb (fp8 DoubleRow) --------------
            g_bf = [persist.tile([128, C], BF16, name=f"gbf{i}")
                    for i in range(2)]
            wmt = persist.tile([128, 256], BF16, name="wmt")
            nc.gpsimd.memset(wmt[:], 0.125)
            nmm = T // 2
            # PE p-state ramps to full clock only after ~3us of continuous
            # execution; burn the initial DMA wait on junk LDWEIGHTS (no
            # PSUM, no memset dependency beyond wmt) so G runs ramped.
            for w in range(10):
                nc.tensor.ldweights(wmt[:, 0:128])
            with tc.tile_pool(name="psg", bufs=1, space="PSUM") as psg:
                g_ps = [psg.tile([128, C], F32, name=f"g{i}")
                        for i in range(2)]
                u0 = 0
                for q, nb in enumerate(PIECES):
                    e3 = emb_t[q][:].rearrange("p (t c) -> p t c", c=C)
                    for u in range(nb // 2):
                        mv = e3[:, 2 * u:2 * u + 2, :]
                        for ch in range(2):
                            nc.tensor.matmul(
                                g_ps[ch][:],
                                e3[:, 2 * u:2 * u + 2,
                                   ch * 128:(ch + 1) * 128],
                                mv,
                                start=(u0 == 0), stop=(u0 == nmm - 1),
                                perf_mode=PM.DoubleRow)
                        u0 += 1
                for ch in range(2):
                    nc.vector.tensor_copy(g_bf[ch][:], g_ps[ch][:])

            pswork_cm = tc.tile_pool(name="pswork", bufs=1, space="PSUM")
            pswork = pswork_cm.__enter__()
            p_ps = [pswork.tile([128, C], F32, name=f"pps{i}",
                                tag=f"pps{i}")[:] for i in range(2)]
            def junk(n):
                # keep the PE p-state ramped while V/S-bound stages run:
                # an idle gap drops the PE clock from 2.4 to 1.2 GHz and
                # triples every subsequent chain matmul.  Junk LDWEIGHTS
                # busy the array without touching PSUM.
                for _ in range(n):
                    nc.tensor.ldweights(wmt[:, 0:128])

            # ---------------- U = G @ Wk ----------------
            u_bf = [persist.tile([128, C * H], BF16, name=f"ubf{i}")
                    for i in range(2)]
            ucopy = [nc.scalar.copy, nc.vector.tensor_copy]

            def emit_u(f):
                for mh in range(2):
                    u_ps = pswork.tile([128, 512], F32, name="ups", tag="ups",
                                       bufs=2)
                    for kc in range(2):
                        nc.tensor.matmul(
                            u_ps[:],
                            g_bf[kc][:, mh * 128:(mh + 1) * 128],
                            wk_sb[:, (f * 2 + kc) * 512:
                                  (f * 2 + kc + 1) * 512],
                            start=(kc == 0), stop=(kc == 1))
                    ucopy[mh](u_bf[mh][:, f * 512:(f + 1) * 512], u_ps[:])

            # -------- head chain, software-pipelined over pairs ----------
            # Stages per pair: A = A-matmuls + bn_stats; B = stats
            # aggregation -> rinv (partition_all_reduce on GpSimd, no
            # PE/PSUM involved); C = exp/softmax -> M; D = M drain + P.
            # Emission A0,B0,A1,B1,C0,A2,B2,D0,C1,A3,B3,D1,C2,D2,C3,D3
            # keeps every in-order engine queue supplied with ready work.
            import concourse.bass_isa as bass_isa
            NP = H // 2
            state = {}
            rdbg_sb = None
            if dbg:
                rdbg_sb = persist.tile([128, 16], F32, name="rdbg_sb")

            def st_a(pr, only=None):
                if only is None or only == 0:
                    a_t = []
                    bnt = chain.tile([128, 24], F32, name="bnt", tag="bnt",
                                     bufs=2)
                    state[pr] = [a_t, bnt, None]
                else:
                    a_t, bnt, _ = state[pr]
                heads = ((2 * pr, 2 * pr + 1) if only is None
                         else (2 * pr + only,))
                for off, h in enumerate(heads):
                    i = off if only is None else only
                    at = pswork.tile([128, 2 * C], F32, name="aps",
                                     tag="work", bufs=4)
                    for dh in range(2):
                        for kc in range(2):
                            nc.tensor.matmul(
                                at[:, dh * C:(dh + 1) * C],
                                wq_sb[:, (h * 2 + kc) * C + dh * 128:
                                      (h * 2 + kc) * C + (dh + 1) * 128],
                                u_bf[kc][:, h * C:(h + 1) * C],
                                start=(kc == 0), stop=(kc == 1))
                        nc.vector.bn_stats(
                            bnt[:, 12 * i + 6 * dh:12 * i + 6 * dh + 6],
                            at[:, dh * C:(dh + 1) * C])
                    a_t.append(at)
                junk(3 if only is not None else 5)

            def st_b(pr):
                a_t, bnt, _ = state[pr]
                # per-head (mean, var) over 512 elems -> statc
                # [4pr..4pr+3] = [mean_h0, E[x^2]_h0, mean_h1, E[x^2]_h1]
                for i in range(2):
                    nc.vector.bn_aggr(statc[:, 4 * pr + 2 * i:
                                            4 * pr + 2 * i + 2],
                                      bnt[:, 12 * i:12 * i + 12])
                musq = chain.tile([128, 2], F32, name="musq", tag="musq",
                                  bufs=2)
                nc.gpsimd.tensor_mul(musq[:], statc[:, 4 * pr:4 * pr + 4:2],
                                     statc[:, 4 * pr:4 * pr + 4:2])
                nc.gpsimd.tensor_add(statc[:, 4 * pr + 1:4 * pr + 4:2],
                                     statc[:, 4 * pr + 1:4 * pr + 4:2],
                                     musq[:])
                st_ps = pswork.tile([128, 4], F32, name="stps", tag="ups",
                                    bufs=2)
                nc.tensor.matmul(st_ps[:], ones[:],
                                 statc[:, 4 * pr:4 * pr + 4],
                                 start=True, stop=True)
                stbc = chain.tile([128, 4], F32, name="stbc", tag="stbc",
                                  bufs=2)
                nc.vector.tensor_copy(stbc[:], st_ps[:])
                mue2 = chain.tile([128, 4], F32, name="mue2", tag="mue2",
                                  bufs=2)
                nc.gpsimd.tensor_scalar_mul(mue2[:], stbc[:], 1.0 / 128.0)
                musq2 = chain.tile([128, 2], F32, name="musq2", tag="musq2",
                                   bufs=2)
                nc.gpsimd.tensor_mul(musq2[:], mue2[:, 0::2], mue2[:, 0::2])
                var = chain.tile([128, 2], F32, name="var", tag="var", bufs=2)
                nc.gpsimd.tensor_sub(var[:], mue2[:, 1::2], musq2[:])
                lv = chain.tile([128, 2], F32, name="lv", tag="lv", bufs=2)
                nc.scalar.activation(lv[:], var[:], AF.Ln, bias=epst[:])
                rinv = chain.tile([128, 2], F32, name="rinv", tag="rinv",
                                  bufs=2)
                nc.scalar.activation(rinv[:], lv[:], AF.Exp, scale=-0.5)
                junk(4)
                if dbg:
                    nc.vector.tensor_copy(rdbg_sb[:, 2 * pr:2 * pr + 2],
                                          rinv[:])
                    nc.vector.tensor_copy(rdbg_sb[:, 8 + 2 * pr:10 + 2 * pr],
                                          var[:])
                state[pr][2] = rinv

            def st_c(pr):
                a_t, bnt, rinv = state[pr]
                ets = []
                for i, h in enumerate((2 * pr, 2 * pr + 1)):
                    for dh in range(2):
                        e_t = ebuf.tile([128, C], BF16, name="et", tag="et",
                                        bufs=8)
                        nc.scalar.activation(
                            e_t[:], a_t[i][:, dh * C:(dh + 1) * C], AF.Exp,
                            scale=rinv[:, i:i + 1],
                            accum_out=esum[:, 2 * h + dh:2 * h + dh + 1])
                        ets.append(e_t)
                nc.vector.reciprocal(rec[:, 4 * pr:4 * pr + 4],
                                     esum[:, 4 * pr:4 * pr + 4])
                for i, h in enumerate((2 * pr, 2 * pr + 1)):
                    et = ets[2 * i:2 * i + 2]
                    woh = []
                    for dh in range(2):
                        w_t = chain.tile([128, C], BF16, name="woh",
                                         tag="woh", bufs=8)
                        nc.vector.tensor_scalar_mul(
                            w_t[:],
                            wvo_sb[:, (h * 4 + dh) * C:(h * 4 + dh + 1) * C],
                            rec[:, 2 * h + dh:2 * h + dh + 1])
                        woh.append(w_t)
                    m_t = pswork.tile([128, 512], F32, name="mps", tag="ups",
                                      bufs=2)
                    for eh in range(2):
                        for dh in range(2):
                            nc.tensor.matmul(
                                m_t[:, eh * C:(eh + 1) * C],
                                et[dh][:, eh * 128:(eh + 1) * 128],
                                woh[dh][:],
                                start=(dh == 0), stop=(dh == 1))
                    state[pr].append(m_t)
                junk(5)

            def st_d(pr):
                ent = state.pop(pr)
                m_ts = ent[3:5]
                for i, h in enumerate((2 * pr, 2 * pr + 1)):
                    m_bf = chain.tile([128, 512], BF16, name="mbf",
                                      tag="mbf", bufs=2)
                    nc.vector.tensor_copy(m_bf[:], m_ts[i][:])
                    for ch in range(2):
                        for eh in range(2):
                            nc.tensor.matmul(
                                p_ps[ch],
                                wvo_sb[:, (h * 4 + 2 + eh) * C + ch * 128:
                                       (h * 4 + 2 + eh) * C + (ch + 1) * 128],
                                m_bf[:, eh * C:(eh + 1) * C],
                                start=(h == 0 and eh == 0),
                                stop=(h == H - 1 and eh == 1))
                junk(5)

            emit_u(0); st_a(0); emit_u(1); st_a(1); st_b(0)
            emit_u(2); st_b(1); emit_u(3)
            st_a(2, only=0)
            st_c(0); st_d(0); st_a(2, only=1); st_b(2)
            st_a(3, only=0)
            st_c(1); st_d(1); st_a(3, only=1); st_b(3)
            st_c(2); st_d(2); st_c(3); st_d(3)

            p_sb = [persist.tile([128, C], BF16, name=f"psb{i}")
                    for i in range(2)]
            nc.vector.tensor_copy(p_sb[0][:], p_ps[0])
            nc.scalar.copy(p_sb[1][:], p_ps[1])
            pswork_cm.__exit__(None, None, None)

            # ---------------- outT = P^T @ embT ----------------
            psout_cm = tc.tile_pool(name="psout", bufs=1, space="PSUM")
            psout = psout_cm.__enter__()
            ocopy = [nc.vector.tensor_copy, nc.scalar.copy]
            k = 0
            for nb in range(CHUNK // 512):
                ns = slice(nb * 512, (nb + 1) * 512)
                for ch in range(2):
                    o_ps = psout.tile([128, 512], F32, name="ops", tag="ops",
                                      bufs=4)
                    for i in range(2):
                        nc.tensor.matmul(
                            o_ps[:],
                            p_sb[i][:, ch * 128:(ch + 1) * 128],
                            embt_sb[:, i * CHUNK + nb * 512:
                                    i * CHUNK + (nb + 1) * 512],
                            start=(i == 0), stop=(i == 1))
                    o_sb = chain.tile([128, 512], BF16, name="osb", tag="osb",
                                      bufs=4)
                    ocopy[k % 2](o_sb[:], o_ps[:])
                    k += 1
                    nc.sync.dma_start(outt[ch * 128:(ch + 1) * 128, ns],
                                      o_sb[:])
            psout_cm.__exit__(None, None, None)

            if dbg:
                for ch in range(2):
                    nc.sync.dma_start(
                        dbg_t["gdbg"][:, ch * C:(ch + 1) * C], g_bf[ch][:])
                    nc.sync.dma_start(
                        dbg_t["udbg"][:, ch * 2048:(ch + 1) * 2048],
                        u_bf[ch][:])
                    nc.sync.dma_start(
                        dbg_t["pdbg"][:, ch * C:(ch + 1) * C], p_sb[ch][:])
                nc.sync.dma_start(dbg_t["sdbg"][:, 0:16], statc[:])
                nc.sync.dma_start(dbg_t["edbg"][:], esum[:])
                nc.sync.dma_start(dbg_t["rdbg"][:], rdbg_sb[:])

    nc.compile()
    return nc


_NC_CACHE = None


def kernel(emb, Wq, Wk, Wv, Wo):
    global _NC_CACHE
    emb = np.ascontiguousarray(np.asarray(emb, dtype=np.float32))
    Wq = np.ascontiguousarray(np.asarray(Wq, dtype=np.float32))
    Wk = np.ascontiguousarray(np.asarray(Wk, dtype=np.float32))
    Wv = np.ascontiguousarray(np.asarray(Wv, dtype=np.float32))
    Wo = np.ascontiguousarray(np.asarray(Wo, dtype=np.float32))

    if _NC_CACHE is None:
        _NC_CACHE = build_kernel()
    nc = _NC_CACHE

    bf = ml_dtypes.bfloat16
    f8 = ml_dtypes.float8_e4m3

    wkt = np.ascontiguousarray(
        Wk.reshape(2, 128, 4, 512).transpose(1, 2, 0, 3)
        .reshape(128, 4096)).astype(bf)
    wqt = np.ascontiguousarray(
        Wq.reshape(2, 128, 8, 256).transpose(1, 2, 0, 3)
        .reshape(128, 4096)).astype(bf)
    wos4 = Wo.reshape(2, 128, 8, 256).transpose(1, 2, 0, 3)  # [p,h,dh,c]
    wvt4 = np.ascontiguousarray(Wv.T).reshape(8, 2, 128, 256) \
        .transpose(2, 0, 1, 3)                               # [p,h,eh,x]
    wvo = np.ascontiguousarray(
        np.concatenate([wos4, wvt4], axis=2).reshape(128, 8192)).astype(bf)

    emb8 = [np.ascontiguousarray(emb[b].reshape(128, T * C)).astype(f8)
            for b in range(B)]

    in_maps = []
    for c in range(NCORES):
        b, j = divmod(c, NCORES // B)
        ec = emb[b][j * CHUNK:(j + 1) * CHUNK, :]
        etp = np.ascontiguousarray(
            ec.T.reshape(2, 128, CHUNK).transpose(1, 0, 2)
            .reshape(128, 2 * CHUNK)).astype(bf)
        in_maps.append({
            "emb8": emb8[b], "wkt": wkt, "wqt": wqt, "wvo": wvo,
            "embt": etp,
        })

    trace = bool(int(os.environ.get("KERNEL_TRACE", "0")))
    res = run_bass_kernel_spmd(nc, in_maps, core_ids=list(range(NCORES)),
                               trace=trace)
    kernel.last_result = res

    full = np.empty((B, N, C), dtype=np.float32)
    for c in range(NCORES):
        b, j = divmod(c, NCORES // B)
        full[b, j * CHUNK:(j + 1) * CHUNK, :] = \
            res.results[c]["outt"].astype(np.float32).T
    return full


# revision 38
# speedup vs baseline: 1.1371x; 1.1371x over previous
"""Trainium2 Bass kernel for nn_Attn_66297115181215 (sparse_attention).

Reference computation (B=2, N=8192, C=256, H=8, Dh=C):
    qh/kh/vh = heads(emb @ W{q,k,v})            [B,H,N,Dh]
    attn = einsum("bhnd,bhne->bhde", qh, kh)    [B,H,Dh,Dh]
    attn = instance_norm(attn); attn = softmax(attn, axis=3)
    ctx  = einsum("bhde,bhne->bhdn", attn, vh)  [B,H,Dh,N]
    out  = ctx.transpose(0,3,2,1).reshape(B,N,C*H) @ Wo

Algebraic collapse: N only enters through G_b = emb_b^T emb_b [C,C]:
    A_h = Wq_h^T G Wk_h;  S_h = softmax(instnorm(A_h))
    out_b = emb_b @ P_b,  P_b = sum_h Wv_h (S_h^T Wo'_h)
(the softmax denominator is folded into Wo'_h rows).

Distribution (8 cores, no collectives): core c: b = c//4, j = c%4.
Every core redundantly computes G_b and the 8-head chain, then its own
N/4 slice of out_b (transposed; host transposes back).

Precision: inputs are cast host-side.  emb arrives as fp8(e4m3) and G
runs fp8 DoubleRow matmuls (fp32 accumulate).  The head chain and out
projection run bf16 (fp32 accumulate).  Output is written bf16 and
upcast on host.  End-to-end max-rel error ~6.5e-3 vs the 2e-2 gate.

Engine/latency structure:
 - Scalar/ACT stays on the single natural_log_exp table (rinv =
   exp(-0.5*ln(var+eps)); a table switch costs 1.3us).  The -mu shift
   cancels in softmax and is skipped.
 - The head chain is software-pipelined over head PAIRS with a lag of
   one pair, so each engine's in-order queue always has ready work:
   PH1 = A matmuls + stats -> rinv; PH2 = exp/softmax -> M -> P.
 - Warm-up matmuls burn the initial DMA wait so the PE p-state ramp
   (full clock only after ~3us of continuous work) is done before G.
 - DMA issue order = use order (emb pieces, Wk, Wq, Wv/Wo, embT).
"""

import os
import sys

sys.path.insert(0, "/opt/trn_rl_repo")

import ml_dtypes
import numpy as np

import concourse.bacc as bacc
import concourse.mybir as mybir
import concourse.tile as tile
from concourse.bass_utils import run_bass_kernel_spmd

# The kernel only uses Exp/Ln/Copy on the ACT engine, all of which live in
# the natural_log_exp_and_others table.  The default table-selection pass
# assigns Exp and Ln to different tables and thrashes (1283ns per load).
# Castrate every other table (preserving list positions, which are the
# act_func_set_id namespace) so one table serves the whole kernel.
_ORIG_GAT = bacc.get_activation_tables


def _single_act_table(arch):
    t = _ORIG_GAT(arch)
    return {name: (funcs if name == "natural_log_exp_and_others" else set())
            for name, funcs in t.items()}


bacc.get_activation_tables = _single_act_table

B, N, C, H = 2, 8192, 256, 8
EPS = 1e-5
NCORES = 8
CHUNK = N * B // NCORES          # 2048 out rows per core
T = N // 128                     # 64 row-blocks of emb

F32 = mybir.dt.float32
F32R = mybir.dt.float32r
BF16 = mybir.dt.bfloat16
F8 = mybir.dt.float8e4
AF = mybir.ActivationFunctionType
ALU = mybir.AluOpType
AX = mybir.AxisListType
PM = mybir.MatmulPerfMode

# emb piece sizes in t-blocks (all even so fp8 DoubleRow pairs never
# straddle pieces; big first so the tail G matmuls are tiny)
PIECES = [4, 8, 16, 16, 12, 6, 2]
assert sum(PIECES) == T and all(p % 2 == 0 for p in PIECES)


def build_kernel(dbg=False):
    nc = bacc.Bacc("TRN2", target_bir_lowering=False, debug=False,
                   num_devices=NCORES)
    dbg_t = {}
    if dbg:
        for nm, cols, dt in [("gdbg", 512, BF16), ("udbg", 4096, BF16),
                             ("sdbg", 32, F32), ("edbg", 16, F32),
                             ("rdbg", 16, F32), ("pdbg", 512, BF16)]:
            dbg_t[nm] = nc.dram_tensor(nm, [128, cols], dt,
                                       kind="ExternalOutput")

    # [p, t, c]: col t*C+c = emb row p*64+t (row-major reshape)
    emb8 = nc.dram_tensor("emb8", [128, T * C], F8, kind="ExternalInput")
    # [p, f, kc, n]: Wk[kc*128+p, f*512+n]
    wkt = nc.dram_tensor("wkt", [128, 8 * 512], BF16, kind="ExternalInput")
    # [p, h, kc, d]: Wq[kc*128+p, h*256+d]
    wqt = nc.dram_tensor("wqt", [128, 16 * C], BF16, kind="ExternalInput")
    # [p, h, s, x]: s=0,1 -> Wo[(s*128+p)*H+h, x]; s=2,3 -> Wv[x, h*C+(s-2)*128+p]
    wvo = nc.dram_tensor("wvo", [128, 32 * C], BF16, kind="ExternalInput")
    # [p, i, n]: embT row i*128+p of this core's chunk
    embt = nc.dram_tensor("embt", [128, 2 * CHUNK], BF16, kind="ExternalInput")
    outt = nc.dram_tensor("outt", [C, CHUNK], BF16, kind="ExternalOutput")

    with tile.TileContext(nc) as tc:
        with (
            tc.tile_pool(name="wbuf", bufs=1) as wbuf,
            tc.tile_pool(name="persist", bufs=1) as persist,
            tc.tile_pool(name="ebuf", bufs=1) as ebuf,
            tc.tile_pool(name="chain", bufs=1) as chain,
        ):
            # ---------------- DMAs, in use order ----------------
            emb_t = []
            off = 0
            for q, nb in enumerate(PIECES):
                s = persist.tile([128, nb * C], F8, name=f"e{q}")
                nc.sync.dma_start(s[:], emb8[:, off * C:(off + nb) * C])
                emb_t.append(s)
                off += nb
            wk_sb = wbuf.tile([128, 8 * 512], BF16, name="wk")
            nc.sync.dma_start(wk_sb[:], wkt[:])
            wq_sb = wbuf.tile([128, 16 * C], BF16, name="wq")
            nc.sync.dma_start(wq_sb[:], wqt[:])
            wvo_sb = wbuf.tile([128, 32 * C], BF16, name="wvo")
            nc.sync.dma_start(wvo_sb[:], wvo[:])
            embt_sb = wbuf.tile([128, 2 * CHUNK], BF16, name="embt")
            nc.sync.dma_start(embt_sb[:], embt[:])

            ones = persist.tile([128, 128], F32, name="ones")
            nc.gpsimd.memset(ones[:], 1.0)
            epst = persist.tile([128, 1], F32, name="epst")
            nc.gpsimd.memset(epst[:], EPS)
            # dummy activation: loads the (single) ACT table during the
            # initial DMA wait instead of at the first real Ln (1283ns
            # on the chain critical path otherwise)
            twarm = persist.tile([128, 1], F32, name="twarm")
            nc.scalar.activation(twarm[:], epst[:], AF.Exp)
            # st_sb per pair = [Ssum_h0, Ssum_h1, Ssq_h0, Ssq_h1]; the sum
            # cols hold per-partition (mean_even+mean_odd) = rowsum/256
            # over the 512-wide a_t tile
            scale4 = persist.tile([128, 4], F32, name="scale4")
            nc.gpsimd.memset(scale4[:, 0:2], 1.0 / 256.0)
            nc.gpsimd.memset(scale4[:, 2:4], 1.0 / float(C * C))
            # per-pair stats cols [4p..4p+3] = [s_h0, s_h1, q_h0, q_h1]
            statc = persist.tile([128, 2 * H], F32, name="statc")
            esum = persist.tile([128, 2 * H], F32, name="esum")
            rec = persist.tile([128, 2 * H], F32, name="rec")

            # ---------------- G = emb^T emb (fp8 DoubleRow) --------------
            g_bf = [persist.tile([128, C], BF16, name=f"gbf{i}")
                    for i in range(2)]
            wmt = persist.tile([128, 256], BF16, name="wmt")
            nc.gpsimd.memset(wmt[:], 0.125)
            nmm = T // 2
            # PE p-state ramps to full clock only after ~3us of continuous
            # execution; burn the initial DMA wait on junk LDWEIGHTS (no
            # PSUM, no memset dependency beyond wmt) so G runs ramped.
            for w in range(10):
                nc.tensor.ldweights(wmt[:, 0:128])
            with tc.tile_pool(name="psg", bufs=1, space="PSUM") as psg:
                g_ps = [psg.tile([128, C], F32, name=f"g{i}")
                        for i in range(2)]
                u0 = 0
                for q, nb in enumerate(PIECES):
                    e3 = emb_t[q][:].rearrange("p (t c) -> p t c", c=C)
                    for u in range(nb // 2):
                        mv = e3[:, 2 * u:2 * u + 2, :]
                        for ch in range(2):
                            nc.tensor.matmul(
                                g_ps[ch][:],
                                e3[:, 2 * u:2 * u + 2,
                                   ch * 128:(ch + 1) * 128],
                                mv,
                                start=(u0 == 0), stop=(u0 == nmm - 1),
                                perf_mode=PM.DoubleRow)
                        u0 += 1
                for ch in range(2):
                    nc.vector.tensor_copy(g_bf[ch][:], g_ps[ch][:])

            pswork_cm = tc.tile_pool(name="pswork", bufs=1, space="PSUM")
            pswork = pswork_cm.__enter__()
            p_ps = [pswork.tile([128, C], F32, name=f"pps{i}",
                                tag=f"pps{i}")[:] for i in range(2)]
            def junk(n):
                # keep the PE p-state ramped while V/S-bound stages run:
                # an idle gap drops the PE clock from 2.4 to 1.2 GHz and
                # triples every subsequent chain matmul.  Junk LDWEIGHTS
                # busy the array without touching PSUM.
                for _ in range(n):
                    nc.tensor.ldweights(wmt[:, 0:128])

            # ---------------- U = G @ Wk ----------------
            u_bf = [persist.tile([128, C * H], BF16, name=f"ubf{i}")
                    for i in range(2)]
            ucopy = [nc.scalar.copy, nc.vector.tensor_copy]

            def emit_u(f):
                for mh in range(2):
                    u_ps = pswork.tile([128, 512], F32, name="ups", tag="ups",
                                       bufs=2)
                    for kc in range(2):
                        nc.tensor.matmul(
                            u_ps[:],
                            g_bf[kc][:, mh * 128:(mh + 1) * 128],
                            wk_sb[:, (f * 2 + kc) * 512:
                                  (f * 2 + kc + 1) * 512],
                            start=(kc == 0), stop=(kc == 1))
                    ucopy[mh](u_bf[mh][:, f * 512:(f + 1) * 512], u_ps[:])

            # -------- head chain, software-pipelined over pairs ----------
            # Stages per pair: A = A-matmuls + bn_stats; B = stats
            # aggregation -> rinv (partition_all_reduce on GpSimd, no
            # PE/PSUM involved); C = exp/softmax -> M; D = M drain + P.
            # Emission A0,B0,A1,B1,C0,A2,B2,D0,C1,A3,B3,D1,C2,D2,C3,D3
            # keeps every in-order engine queue supplied with ready work.
            import concourse.bass_isa as bass_isa
            NP = H // 2
            state = {}
            rdbg_sb = None
            if dbg:
                rdbg_sb = persist.tile([128, 16], F32, name="rdbg_sb")

            def st_a(pr, only=None):
                if only is None or only == 0:
                    a_t = []
                    bnt = chain.tile([128, 24], F32, name="bnt", tag="bnt",
                                     bufs=2)
                    state[pr] = [a_t, bnt, None]
                else:
                    a_t, bnt, _ = state[pr]
                heads = ((2 * pr, 2 * pr + 1) if only is None
                         else (2 * pr + only,))
                for off, h in enumerate(heads):
                    i = off if only is None else only
                    at = pswork.tile([128, 2 * C], F32, name="aps",
                                     tag="work", bufs=4)
                    for dh in range(2):
                        for kc in range(2):
                            nc.tensor.matmul(
                                at[:, dh * C:(dh + 1) * C],
                                wq_sb[:, (h * 2 + kc) * C + dh * 128:
                                      (h * 2 + kc) * C + (dh + 1) * 128],
                                u_bf[kc][:, h * C:(h + 1) * C],
                                start=(kc == 0), stop=(kc == 1))
                        nc.vector.bn_stats(
                            bnt[:, 12 * i + 6 * dh:12 * i + 6 * dh + 6],
                            at[:, dh * C:(dh + 1) * C])
                    a_t.append(at)
                junk(3 if only is not None else 5)

            def st_b(pr):
                a_t, bnt, _ = state[pr]
                # per-head (mean, var) over 512 elems -> statc
                # [4pr..4pr+3] = [mean_h0, E[x^2]_h0, mean_h1, E[x^2]_h1]
                for i in range(2):
                    nc.vector.bn_aggr(statc[:, 4 * pr + 2 * i:
                                            4 * pr + 2 * i + 2],
                                      bnt[:, 12 * i:12 * i + 12])
                musq = chain.tile([128, 2], F32, name="musq", tag="musq",
                                  bufs=2)
                nc.gpsimd.tensor_mul(musq[:], statc[:, 4 * pr:4 * pr + 4:2],
                                     statc[:, 4 * pr:4 * pr + 4:2])
                nc.gpsimd.tensor_add(statc[:, 4 * pr + 1:4 * pr + 4:2],
                                     statc[:, 4 * pr + 1:4 * pr + 4:2],
                                     musq[:])
                st_ps = pswork.tile([128, 4], F32, name="stps", tag="ups",
                                    bufs=2)
                nc.tensor.matmul(st_ps[:], ones[:],
                                 statc[:, 4 * pr:4 * pr + 4],
                                 start=True, stop=True)
                stbc = chain.tile([128, 4], F32, name="stbc", tag="stbc",
                                  bufs=2)
                nc.vector.tensor_copy(stbc[:], st_ps[:])
                mue2 = chain.tile([128, 4], F32, name="mue2", tag="mue2",
                                  bufs=2)
                nc.gpsimd.tensor_scalar_mul(mue2[:], stbc[:], 1.0 / 128.0)
                musq2 = chain.tile([128, 2], F32, name="musq2", tag="musq2",
                                   bufs=2)
                nc.gpsimd.tensor_mul(musq2[:], mue2[:, 0::2], mue2[:, 0::2])
                var = chain.tile([128, 2], F32, name="var", tag="var", bufs=2)
                nc.gpsimd.tensor_sub(var[:], mue2[:, 1::2], musq2[:])
                lv = chain.tile([128, 2], F32, name="lv", tag="lv", bufs=2)
                nc.scalar.activation(lv[:], var[:], AF.Ln, bias=epst[:])
                rinv = chain.tile([128, 2], F32, name="rinv", tag="rinv",
                                  bufs=2)
                nc.scalar.activation(rinv[:], lv[:], AF.Exp, scale=-0.5)
                junk(4)
                if dbg:
                    nc.vector.tensor_copy(rdbg_sb[:, 2 * pr:2 * pr + 2],
                                          rinv[:])
                    nc.vector.tensor_copy(rdbg_sb[:, 8 + 2 * pr:10 + 2 * pr],
                                          var[:])
                state[pr][2] = rinv

            def st_c(pr):
                a_t, bnt, rinv = state[pr]
                ets = []
                for i, h in enumerate((2 * pr, 2 * pr + 1)):
                    for dh in range(2):
                        e_t = ebuf.tile([128, C], BF16, name="et", tag="et",
                                        bufs=8)
                        nc.scalar.activation(
                            e_t[:], a_t[i][:, dh * C:(dh + 1) * C], AF.Exp,
                            scale=rinv[:, i:i + 1],
                            accum_out=esum[:, 2 * h + dh:2 * h + dh + 1])
                        ets.append(e_t)
                nc.vector.reciprocal(rec[:, 4 * pr:4 * pr + 4],
                                     esum[:, 4 * pr:4 * pr + 4])
                for i, h in enumerate((2 * pr, 2 * pr + 1)):
                    et = ets[2 * i:2 * i + 2]
                    woh = []
                    for dh in range(2):
                        w_t = chain.tile([128, C], BF16, name="woh",
                                         tag="woh", bufs=8)
                        nc.vector.tensor_scalar_mul(
                            w_t[:],
                            wvo_sb[:, (h * 4 + dh) * C:(h * 4 + dh + 1) * C],
                            rec[:, 2 * h + dh:2 * h + dh + 1])
                        woh.append(w_t)
                    m_t = pswork.tile([128, 512], F32, name="mps", tag="ups",
                                      bufs=2)
                    for eh in range(2):
                        for dh in range(2):
                            nc.tensor.matmul(
                                m_t[:, eh * C:(eh + 1) * C],
                                et[dh][:, eh * 128:(eh + 1) * 128],
                                woh[dh][:],
                                start=(dh == 0), stop=(dh == 1))
                    state[pr].append(m_t)
                junk(5)

            def st_d(pr):
                ent = state.pop(pr)
                m_ts = ent[3:5]
                for i, h in enumerate((2 * pr, 2 * pr + 1)):
                    m_bf = chain.tile([128, 512], BF16, name="mbf",
                                      tag="mbf", bufs=2)
                    nc.vector.tensor_copy(m_bf[:], m_ts[i][:])
                    for ch in range(2):
                        for eh in range(2):
                            nc.tensor.matmul(
                                p_ps[ch],
                                wvo_sb[:, (h * 4 + 2 + eh) * C + ch * 128:
                                       (h * 4 + 2 + eh) * C + (ch + 1) * 128],
                                m_bf[:, eh * C:(eh + 1) * C],
                                start=(h == 0 and eh == 0),
                                stop=(h == H - 1 and eh == 1))
                junk(5)

            emit_u(0); st_a(0); emit_u(1); st_a(1); st_b(0)
            emit_u(2); st_b(1); emit_u(3)
            st_c(0); st_d(0); st_a(2); st_b(2)
            st_c(1); st_d(1); st_a(3); st_b(3)
            st_c(2); st_d(2); st_c(3); st_d(3)

            p_sb = [persist.tile([128, C], BF16, name=f"psb{i}")
                    for i in range(2)]
            nc.vector.tensor_copy(p_sb[0][:], p_ps[0])
            nc.scalar.copy(p_sb[1][:], p_ps[1])
            pswork_cm.__exit__(None, None, None)

            # ---------------- outT = P^T @ embT ----------------
            psout_cm = tc.tile_pool(name="psout", bufs=1, space="PSUM")
            psout = psout_cm.__enter__()
            ocopy = [nc.vector.tensor_copy, nc.scalar.copy]
            k = 0
            for nb in range(CHUNK // 512):
                ns = slice(nb * 512, (nb + 1) * 512)
                for ch in range(2):
                    o_ps = psout.tile([128, 512], F32, name="ops", tag="ops",
                                      bufs=4)
                    for i in range(2):
                        nc.tensor.matmul(
                            o_ps[:],
                            p_sb[i][:, ch * 128:(ch + 1) * 128],
                            embt_sb[:, i * CHUNK + nb * 512:
                                    i * CHUNK + (nb + 1) * 512],
                            start=(i == 0), stop=(i == 1))
                    o_sb = chain.tile([128, 512], BF16, name="osb", tag="osb",
                                      bufs=4)
                    ocopy[k % 2](o_sb[:], o_ps[:])
                    k += 1
                    nc.sync.dma_start(outt[ch * 128:(ch + 1) * 128, ns],
                                      o_sb[:])
            psout_cm.__exit__(None, None, None)

            if dbg:
                for ch in range(2):
                    nc.sync.dma_start(
                        dbg_t["gdbg"][:, ch * C:(ch + 1) * C], g_bf[ch][:])
                    nc.sync.dma_start(
                        dbg_t["udbg"][:, ch * 2048:(ch + 1) * 2048],
                        u_bf[ch][:])
                    nc.sync.dma_start(
                        dbg_t["pdbg"][:, ch * C:(ch + 1) * C], p_sb[ch][:])
                nc.sync.dma_start(dbg_t["sdbg"][:, 0:16], statc[:])
                nc.sync.dma_start(dbg_t["edbg"][:], esum[:])
                nc.sync.dma_start(dbg_t["rdbg"][:], rdbg_sb[:])

    nc.compile()
    return nc


_NC_CACHE = None


def kernel(emb, Wq, Wk, Wv, Wo):
    global _NC_CACHE
    emb = np.ascontiguousarray(np.asarray(emb, dtype=np.float32))
    Wq = np.ascontiguousarray(np.asarray(Wq, dtype=np.float32))
    Wk = np.ascontiguousarray(np.asarray(Wk, dtype=np.float32))
    Wv = np.ascontiguousarray(np.asarray(Wv, dtype=np.float32))
    Wo = np.ascontiguousarray(np.asarray(Wo, dtype=np.float32))

    if _NC_CACHE is None:
        _NC_CACHE = build_kernel()
    nc = _NC_CACHE

    bf = ml_dtypes.bfloat16
    f8 = ml_dtypes.float8_e4m3

    wkt = np.ascontiguousarray(
        Wk.reshape(2, 128, 4, 512).transpose(1, 2, 0, 3)
        .reshape(128, 4096)).astype(bf)
    wqt = np.ascontiguousarray(
        Wq.reshape(2, 128, 8, 256).transpose(1, 2, 0, 3)
        .reshape(128, 4096)).astype(bf)
    wos4 = Wo.reshape(2, 128, 8, 256).transpose(1, 2, 0, 3)  # [p,h,dh,c]
    wvt4 = np.ascontiguousarray(Wv.T).reshape(8, 2, 128, 256) \
        .transpose(2, 0, 1, 3)                               # [p,h,eh,x]
    wvo = np.ascontiguousarray(
        np.concatenate([wos4, wvt4], axis=2).reshape(128, 8192)).astype(bf)

    emb8 = [np.ascontiguousarray(emb[b].reshape(128, T * C)).astype(f8)
            for b in range(B)]

    in_maps = []
    for c in range(NCORES):
        b, j = divmod(c, NCORES // B)
        ec = emb[b][j * CHUNK:(j + 1) * CHUNK, :]
        etp = np.ascontiguousarray(
            ec.T.reshape(2, 128, CHUNK).transpose(1, 0, 2)
            .reshape(128, 2 * CHUNK)).astype(bf)
        in_maps.append({
            "emb8": emb8[b], "wkt": wkt, "wqt": wqt, "wvo": wvo,
            "embt": etp,
        })

    trace = bool(int(os.environ.get("KERNEL_TRACE", "0")))
    res = run_bass_kernel_spmd(nc, in_maps, core_ids=list(range(NCORES)),
                               trace=trace)
    kernel.last_result = res

    full = np.empty((B, N, C), dtype=np.float32)
    for c in range(NCORES):
        b, j = divmod(c, NCORES // B)
        full[b, j * CHUNK:(j + 1) * CHUNK, :] = \
            res.results[c]["outt"].astype(np.float32).T
    return full
